# revision 1
# baseline (speedup 1.0000x reference)
"""Trainium2 Bass kernel for InvSGSS quantized linear.

out[m, k] = sum_n x[m, n] * W_deq[k, n] + bias[k]
W_deq[k, n] = (W_q[k, n] - zeros[k, g]) * scales[k, g] * mu2[k] * mu1[n],  g = n // 128

Sharding (8 cores): 2 m-shards x 4 k-shards. Each core handles
M_C=4096 rows of x and K_C=1024 output features.

Per-core dataflow:
  Phase 1 (once): DMA W_q shard [128k, N] int32 tiles; dequant on DVE with
    fused tensor_scalar (W*s' + b') where s' = scales*mu2, b' = -zeros*scales*mu2
    (host-folded small tensors); PE-transpose 128x128 chunks to build the
    resident W.T [n, k] bf16 operand, folding mu1[n] in during PSUM evict.
  Phase 2 (streamed): DMA x tiles [128m, N] with fp32->bf16 cast (SWDGE),
    PE-transpose to x.T [n-chunk, m] tiles, then 32 accumulating bf16 matmuls
    per (m-tile, k-tile) psum; bias added on psum evict.
"""

import sys

if "/opt/trn_rl_repo" not in sys.path:
    sys.path.insert(0, "/opt/trn_rl_repo")

import numpy as np

import concourse.bass as bass  # noqa: F401
import concourse.mybir as mybir
import concourse.tile as tile
from concourse import bacc
from concourse.bass_utils import run_bass_kernel_spmd
from concourse.masks import make_identity

K, N = 4096, 4096
GROUP = 128
NG = N // GROUP  # 32 groups along N (group == 128-chunk)
M = 8192  # B*S
B, S = 4, 2048
M_SH, K_SH = 2, 4  # core grid: 2 m-shards x 4 k-shards
MC = M // M_SH  # 4096 rows per core
KC = K // K_SH  # 1024 output features per core
NCH = N // 128  # 32 contraction chunks
MT = MC // 128  # 32 m-tiles
KT = KC // 128  # 8 k-row-tiles of W
KTILE = 512  # matmul free dim (one PSUM bank)
NKT = KC // KTILE  # 2

_CACHE: dict = {}


def build_nc(
    repeat: int = 1,
    debug: bool = False,
    x_cast: str = "act",
    probe: str = "full",
    xT_dma: bool = True,
):
    """x_cast: 'dma' = SWDGE cast-DMA fp32->bf16; 'act' = HWDGE fp32 DMA + ScalarE cast;
    'vec' = HWDGE fp32 DMA + VectorE cast.
    probe: 'full' | 'mm_only' (skip x load/transpose in repeat body) |
    'xprep_only' (skip matmuls in repeat body).
    xT_dma: transpose x tiles via xbar DMA instead of the PE."""
    dt = mybir.dt
    nc = bacc.Bacc("TRN2", target_bir_lowering=False, debug=debug)

    x_d = nc.dram_tensor("x", [MC, N], dt.float32, kind="ExternalInput")
    wq_d = nc.dram_tensor("wq", [KC, N], dt.int32, kind="ExternalInput")
    seff_d = nc.dram_tensor("seff", [KC, NG], dt.float32, kind="ExternalInput")
    beff_d = nc.dram_tensor("beff", [KC, NG], dt.float32, kind="ExternalInput")
    mu1_d = nc.dram_tensor("mu1t", [128, NG], dt.float32, kind="ExternalInput")
    bias_d = nc.dram_tensor("biasb", [128, KC], dt.float32, kind="ExternalInput")
    out_d = nc.dram_tensor("out", [MC, KC], dt.float32, kind="ExternalOutput")

    with tile.TileContext(nc) as tc:
        with tc.tile_pool(name="const", bufs=1) as cpool:
            ident = cpool.tile([128, 128], dt.bfloat16)
            make_identity(nc, ident)
            mu1_sb = cpool.tile([128, NG], dt.float32)
            nc.sync.dma_start(out=mu1_sb, in_=mu1_d[:, :])
            bias_sb = cpool.tile([128, KC], dt.float32)
            nc.sync.dma_start(out=bias_sb, in_=bias_d[:, :])
            seff_sb = cpool.tile([128, KT, NG], dt.float32)
            nc.sync.dma_start(
                out=seff_sb, in_=seff_d.rearrange("(t p) g -> p t g", p=128)
            )
            beff_sb = cpool.tile([128, KT, NG], dt.float32)
            nc.sync.dma_start(
                out=beff_sb, in_=beff_d.rearrange("(t p) g -> p t g", p=128)
            )

            # Resident transposed weight operand: [n % 128, n // 128, k]
            wt_sb = cpool.tile([128, NCH, KC], dt.bfloat16)

            # ---------------- Phase 1: dequant + transpose W ----------------
            with (
                tc.tile_pool(name="wq_pool", bufs=2) as wq_pool,
                tc.tile_pool(name="wstage", bufs=3) as ws_pool,
                tc.tile_pool(name="psw", bufs=2, space="PSUM") as psw_pool,
            ):
                for half in range(2):
                    wq_tiles = []
                    for i in range(4):
                        kt = half * 4 + i
                        wq_t = wq_pool.tile([128, N], dt.int32, name=f"wq_{i}")
                        nc.sync.dma_start(
                            out=wq_t, in_=wq_d[kt * 128 : (kt + 1) * 128, :]
                        )
                        wq_tiles.append((kt, wq_t))
                    for g in range(NG):
                        stage = ws_pool.tile([128, 4, 128], dt.bfloat16, name="wstg")
                        for i, (kt, wq_t) in enumerate(wq_tiles):
                            # (Q * s') + b'  with s' = scales*mu2, b' = -z*s*mu2
                            nc.vector.tensor_scalar(
                                out=stage[:, i, :],
                                in0=wq_t[:, g * 128 : (g + 1) * 128],
                                scalar1=seff_sb[:, kt, g : g + 1],
                                scalar2=beff_sb[:, kt, g : g + 1],
                                op0=mybir.AluOpType.mult,
                                op1=mybir.AluOpType.add,
                            )
                        ps = psw_pool.tile([128, 512], dt.bfloat16, name="psw")
                        for i in range(4):
                            nc.tensor.transpose(
                                ps[:, i * 128 : (i + 1) * 128], stage[:, i, :], ident
                            )
                        # evict with mu1[n] fold (per-partition scalar)
                        nc.vector.tensor_scalar_mul(
                            out=wt_sb[:, g, half * 512 : (half + 1) * 512],
                            in0=ps,
                            scalar1=mu1_sb[:, g : g + 1],
                        )

            # ---------------- Phase 2: stream x, matmul ----------------
            with (
                tc.tile_pool(name="xload", bufs=3) as xl_pool,
                tc.tile_pool(name="xt", bufs=2) as xt_pool,
                tc.tile_pool(name="psx", bufs=2, space="PSUM") as psx_pool,
                tc.tile_pool(name="pso", bufs=4, space="PSUM") as pso_pool,
                tc.tile_pool(name="osb", bufs=4) as osb_pool,
            ):
                def x_prep(mt, pool_tag=""):
                    xb = xl_pool.tile([128, N], dt.bfloat16, name="xb" + pool_tag)
                    if x_cast == "dma":
                        # SWDGE cast DMA: fp32 DRAM -> bf16 SBUF
                        nc.gpsimd.dma_start(
                            out=xb, in_=x_d[mt * 128 : (mt + 1) * 128, :]
                        )
                    else:
                        xf = xl_pool.tile([128, N], dt.float32, name="xf" + pool_tag)
                        nc.sync.dma_start(
                            out=xf, in_=x_d[mt * 128 : (mt + 1) * 128, :]
                        )
                        if x_cast == "act":
                            nc.scalar.copy(out=xb, in_=xf)
                        else:
                            nc.vector.tensor_copy(out=xb, in_=xf)
                    xt_t = xt_pool.tile([128, NCH, 128], dt.bfloat16, name="xt" + pool_tag)
                    if xT_dma:
                        # xbar DMA transpose SBUF->SBUF: [128 m, 4096 n] -> [n%128, n//128, m]
                        nc.scalar.dma_start(out=xt_t[:, :, :], in_=xb[:, :], transpose=True)
                    else:
                        for gb in range(NCH // 4):
                            ps = psx_pool.tile([128, 512], dt.bfloat16, name="psx")
                            for i in range(4):
                                g = gb * 4 + i
                                nc.tensor.transpose(
                                    ps[:, i * 128 : (i + 1) * 128],
                                    xb[:, g * 128 : (g + 1) * 128],
                                    ident,
                                )
                            nc.scalar.copy(
                                out=xt_t[:, gb * 4 : (gb + 1) * 4, :], in_=ps
                            )
                    return xt_t

                xt_fixed = x_prep(0, pool_tag="fix") if probe == "mm_only" else None
                for _rep in range(repeat):
                    for mt in range(MT):
                        if probe == "mm_only":
                            xt_t = xt_fixed
                        else:
                            xt_t = x_prep(mt)
                        if probe == "xprep_only":
                            continue
                        for kt2 in range(NKT):
                            pso = pso_pool.tile([128, KTILE], dt.float32, name="pso")
                            for g in range(NCH):
                                nc.tensor.matmul(
                                    pso,
                                    lhsT=xt_t[:, g, :],
                                    rhs=wt_sb[:, g, kt2 * KTILE : (kt2 + 1) * KTILE],
                                    start=(g == 0),
                                    stop=(g == NCH - 1),
                                )
                            osb = osb_pool.tile([128, KTILE], dt.float32, name="osb")
                            nc.vector.tensor_add(
                                out=osb,
                                in0=pso,
                                in1=bias_sb[:, kt2 * KTILE : (kt2 + 1) * KTILE],
                            )
                            nc.sync.dma_start(
                                out=out_d[
                                    mt * 128 : (mt + 1) * 128,
                                    kt2 * KTILE : (kt2 + 1) * KTILE,
                                ],
                                in_=osb,
                            )
    nc.compile()
    return nc


def make_in_maps(x, W_q, scales, zeros, mu1, mu2, bias):
    x2 = np.ascontiguousarray(np.asarray(x, dtype=np.float32).reshape(M, N))
    W_q = np.asarray(W_q, dtype=np.int32)
    scales = np.asarray(scales, dtype=np.float32).reshape(K, NG)
    zeros = np.asarray(zeros, dtype=np.float32).reshape(K, NG)
    mu1 = np.asarray(mu1, dtype=np.float32)
    mu2 = np.asarray(mu2, dtype=np.float32)
    bias = np.asarray(bias, dtype=np.float32)

    s_eff = scales * mu2[:, None]  # [K, NG]
    b_eff = -(zeros * s_eff)  # [K, NG]
    mu1_t = np.ascontiguousarray(mu1.reshape(NG, 128).T)  # [128, NG]

    in_maps = []
    for c in range(8):
        mi, ki = c // K_SH, c % K_SH
        in_maps.append(
            {
                "x": x2[mi * MC : (mi + 1) * MC],
                "wq": np.ascontiguousarray(W_q[ki * KC : (ki + 1) * KC]),
                "seff": np.ascontiguousarray(s_eff[ki * KC : (ki + 1) * KC]),
                "beff": np.ascontiguousarray(b_eff[ki * KC : (ki + 1) * KC]),
                "mu1t": mu1_t,
                "biasb": np.ascontiguousarray(
                    np.broadcast_to(bias[ki * KC : (ki + 1) * KC], (128, KC))
                ),
            }
        )
    return in_maps


def assemble(results):
    out = np.empty((M, K), np.float32)
    for c in range(8):
        mi, ki = c // K_SH, c % K_SH
        out[mi * MC : (mi + 1) * MC, ki * KC : (ki + 1) * KC] = results[c]["out"]
    return out.reshape(B, S, K)


def kernel(x, W_q, scales, zeros, mu1, mu2, bias):
    in_maps = make_in_maps(x, W_q, scales, zeros, mu1, mu2, bias)
    nc = _CACHE.get("nc")
    if nc is None:
        nc = build_nc()
        _CACHE["nc"] = nc
    res = run_bass_kernel_spmd(nc, in_maps, core_ids=list(range(8)))
    return assemble(res.results)



# revision 3
# speedup vs baseline: 1.3265x; 1.3265x over previous
"""Trainium2 Bass kernel for InvSGSS quantized linear.

out[m, k] = sum_n x[m, n] * W_deq[k, n] + bias[k]
W_deq[k, n] = (W_q[k, n] - zeros[k, g]) * scales[k, g] * mu2[k] * mu1[n],  g = n // 128

Sharding (8 cores): 2 m-shards x 4 k-shards. Each core handles
M_C=4096 rows of x and K_C=1024 output features.

Host prep (layout only): x is pre-blocked per m-shard into
[MT, 128(n%128), NCH*128(m)] fp32 so the device needs no transpose;
W_q is sent as bf16 (values 0..15 are exact); scales/zeros/mu2 folded
into per-(k,group) affine coefficients s' = scales*mu2, b' = -zeros*s'.

Per-core dataflow:
  Phase 1 (once): DMA W bf16 tiles; dequant on DVE with fused
    tensor_scalar (W*s' + b'); PE-transpose 128x128 chunks into the
    resident W.T [n%128, n//128, k] bf16 operand, folding mu1[n] in
    during the PSUM evict (alternating DVE/ACT to spread load).
  Phase 2 (streamed): SWDGE cast-DMA blocked x tiles fp32->bf16 on the
    Pool queue (prefetches during phase 1), then 32 accumulating bf16
    matmuls per (m-tile, k-tile) PSUM bank; bias added on PSUM evict.
"""

import sys

if "/opt/trn_rl_repo" not in sys.path:
    sys.path.insert(0, "/opt/trn_rl_repo")

import numpy as np
from ml_dtypes import bfloat16

import concourse.bass as bass  # noqa: F401
import concourse.mybir as mybir
import concourse.tile as tile
from concourse import bacc
from concourse.bass_utils import run_bass_kernel_spmd
from concourse.masks import make_identity

K, N = 4096, 4096
GROUP = 128
NG = N // GROUP  # 32 groups along N (group == 128-chunk)
M = 8192  # B*S
B, S = 4, 2048
M_SH, K_SH = 2, 4  # core grid: 2 m-shards x 4 k-shards
MC = M // M_SH  # 4096 rows per core
KC = K // K_SH  # 1024 output features per core
NCH = N // 128  # 32 contraction chunks
MT = MC // 128  # 32 m-tiles
KT = KC // 128  # 8 k-row-tiles of W
KTILE = 512  # matmul free dim (one PSUM bank)
NKT = KC // KTILE  # 2

_CACHE: dict = {}


def build_nc(repeat: int = 1, debug: bool = False):
    dt = mybir.dt
    nc = bacc.Bacc("TRN2", target_bir_lowering=False, debug=debug)

    # x blocked on host: [MT, 128 (n%128), NCH*128 (g-major, m-minor)]
    x_d = nc.dram_tensor("x", [MT, 128, N], dt.float32, kind="ExternalInput")
    wq_d = nc.dram_tensor("wq", [KC, N], dt.bfloat16, kind="ExternalInput")
    seff_d = nc.dram_tensor("seff", [KC, NG], dt.float32, kind="ExternalInput")
    beff_d = nc.dram_tensor("beff", [KC, NG], dt.float32, kind="ExternalInput")
    mu1_d = nc.dram_tensor("mu1t", [128, NG], dt.float32, kind="ExternalInput")
    bias_d = nc.dram_tensor("biasb", [128, KC], dt.float32, kind="ExternalInput")
    out_d = nc.dram_tensor("out", [MC, KC], dt.float32, kind="ExternalOutput")

    with tile.TileContext(nc) as tc:
        with (
            tc.tile_pool(name="const", bufs=1) as cpool,
            tc.tile_pool(name="xt", bufs=3) as xt_pool,
            tc.tile_pool(name="pso", bufs=4, space="PSUM") as pso_pool,
            tc.tile_pool(name="osb", bufs=4) as osb_pool,
            tc.tile_pool(name="wq_pool", bufs=2) as wq_pool,
            tc.tile_pool(name="wstage", bufs=3) as ws_pool,
            tc.tile_pool(name="psw", bufs=2, space="PSUM") as psw_pool,
        ):
            ident = cpool.tile([128, 128], dt.bfloat16)
            make_identity(nc, ident)
            mu1_sb = cpool.tile([128, NG], dt.float32)
            nc.sync.dma_start(out=mu1_sb, in_=mu1_d[:, :])
            bias_sb = cpool.tile([128, KC], dt.float32)
            nc.sync.dma_start(out=bias_sb, in_=bias_d[:, :])
            seff_sb = cpool.tile([128, KT, NG], dt.float32)
            nc.sync.dma_start(
                out=seff_sb, in_=seff_d.rearrange("(t p) g -> p t g", p=128)
            )
            beff_sb = cpool.tile([128, KT, NG], dt.float32)
            nc.sync.dma_start(
                out=beff_sb, in_=beff_d.rearrange("(t p) g -> p t g", p=128)
            )

            # Resident transposed weight operand, split per kt2 half:
            # wt[h][n % 128, n // 128, k_local] covers k = h*512 .. h*512+512
            wt_sb = [
                cpool.tile([128, NCH, KTILE], dt.bfloat16, name=f"wt_{h}")
                for h in range(NKT)
            ]

            # ---------------- Phase 1: dequant + transpose W ----------------
            for half in range(NKT):
                wq_tiles = []
                for i in range(4):
                    kt = half * 4 + i
                    wq_t = wq_pool.tile([128, N], dt.bfloat16, name=f"wq_{i}")
                    nc.sync.dma_start(
                        out=wq_t, in_=wq_d[kt * 128 : (kt + 1) * 128, :]
                    )
                    wq_tiles.append((kt, wq_t))
                for g in range(NG):
                    stage = ws_pool.tile([128, 4, 128], dt.bfloat16, name="wstg")
                    for i, (kt, wq_t) in enumerate(wq_tiles):
                        # (Q * s') + b'  with s' = scales*mu2, b' = -z*s*mu2
                        nc.vector.tensor_scalar(
                            out=stage[:, i, :],
                            in0=wq_t[:, g * 128 : (g + 1) * 128],
                            scalar1=seff_sb[:, kt, g : g + 1],
                            scalar2=beff_sb[:, kt, g : g + 1],
                            op0=mybir.AluOpType.mult,
                            op1=mybir.AluOpType.add,
                        )
                    ps = psw_pool.tile([128, KTILE], dt.bfloat16, name="psw")
                    for i in range(4):
                        nc.tensor.transpose(
                            ps[:, i * 128 : (i + 1) * 128], stage[:, i, :], ident
                        )
                    # evict with mu1[n] fold (per-partition scalar);
                    # alternate DVE/ACT so neither engine bottlenecks
                    if g % 2 == 0:
                        nc.vector.tensor_scalar_mul(
                            out=wt_sb[half][:, g, :],
                            in0=ps,
                            scalar1=mu1_sb[:, g : g + 1],
                        )
                    else:
                        nc.scalar.activation(
                            out=wt_sb[half][:, g, :],
                            in_=ps,
                            func=mybir.ActivationFunctionType.Copy,
                            scale=mu1_sb[:, g : g + 1],
                        )

            # ---------------- Phase 2: stream x, matmul ----------------
            for _rep in range(repeat):
                for mt in range(MT):
                    # SWDGE cast-DMA: blocked fp32 DRAM -> bf16 SBUF, already
                    # in [n%128, g, m] layout (host pre-blocked)
                    xt_t = xt_pool.tile([128, NCH, 128], dt.bfloat16, name="xt")
                    nc.gpsimd.dma_start(out=xt_t, in_=x_d[mt])
                    for kt2 in range(NKT):
                        pso = pso_pool.tile([128, KTILE], dt.float32, name="pso")
                        for g in range(NCH):
                            nc.tensor.matmul(
                                pso,
                                lhsT=xt_t[:, g, :],
                                rhs=wt_sb[kt2][:, g, :],
                                start=(g == 0),
                                stop=(g == NCH - 1),
                            )
                        osb = osb_pool.tile([128, KTILE], dt.float32, name="osb")
                        nc.vector.tensor_add(
                            out=osb,
                            in0=pso,
                            in1=bias_sb[:, kt2 * KTILE : (kt2 + 1) * KTILE],
                        )
                        nc.sync.dma_start(
                            out=out_d[
                                mt * 128 : (mt + 1) * 128,
                                kt2 * KTILE : (kt2 + 1) * KTILE,
                            ],
                            in_=osb,
                        )
    nc.compile()
    return nc


def make_in_maps(x, W_q, scales, zeros, mu1, mu2, bias):
    x2 = np.asarray(x, dtype=np.float32).reshape(M, N)
    W_q = np.asarray(W_q, dtype=np.int32)
    scales = np.asarray(scales, dtype=np.float32).reshape(K, NG)
    zeros = np.asarray(zeros, dtype=np.float32).reshape(K, NG)
    mu1 = np.asarray(mu1, dtype=np.float32)
    mu2 = np.asarray(mu2, dtype=np.float32)
    bias = np.asarray(bias, dtype=np.float32)

    s_eff = scales * mu2[:, None]  # [K, NG]
    b_eff = -(zeros * s_eff)  # [K, NG]
    mu1_t = np.ascontiguousarray(mu1.reshape(NG, 128).T)  # [128, NG]
    wq_bf = W_q.astype(bfloat16)  # values 0..15, exact in bf16

    # blocked x per m-shard: [MT, 128(n%128), NCH, 128(m)] -> [MT, 128, N]
    x_blk = []
    for mi in range(M_SH):
        xs = x2[mi * MC : (mi + 1) * MC]  # [MC, N]
        xb = xs.reshape(MT, 128, NCH, 128)  # [mt, m_l, g, p]
        xb = np.ascontiguousarray(xb.transpose(0, 3, 2, 1))  # [mt, p, g, m_l]
        x_blk.append(xb.reshape(MT, 128, N))

    in_maps = []
    for c in range(8):
        mi, ki = c // K_SH, c % K_SH
        in_maps.append(
            {
                "x": x_blk[mi],
                "wq": np.ascontiguousarray(wq_bf[ki * KC : (ki + 1) * KC]),
                "seff": np.ascontiguousarray(s_eff[ki * KC : (ki + 1) * KC]),
                "beff": np.ascontiguousarray(b_eff[ki * KC : (ki + 1) * KC]),
                "mu1t": mu1_t,
                "biasb": np.ascontiguousarray(
                    np.broadcast_to(bias[ki * KC : (ki + 1) * KC], (128, KC))
                ),
            }
        )
    return in_maps


def assemble(results):
    out = np.empty((M, K), np.float32)
    for c in range(8):
        mi, ki = c // K_SH, c % K_SH
        out[mi * MC : (mi + 1) * MC, ki * KC : (ki + 1) * KC] = results[c]["out"]
    return out.reshape(B, S, K)


def kernel(x, W_q, scales, zeros, mu1, mu2, bias):
    in_maps = make_in_maps(x, W_q, scales, zeros, mu1, mu2, bias)
    nc = _CACHE.get("nc")
    if nc is None:
        nc = build_nc()
        _CACHE["nc"] = nc
    res = run_bass_kernel_spmd(nc, in_maps, core_ids=list(range(8)))
    return assemble(res.results)


# revision 20
# speedup vs baseline: 1.4503x; 1.0934x over previous
"""Trainium2 Bass kernel for InvSGSS quantized linear.

out[m, k] = sum_n x[m, n] * W_deq[k, n] + bias[k]
W_deq[k, n] = (W_q[k, n] - zeros[k, g]) * scales[k, g] * mu2[k] * mu1[n],  g = n // 128

Sharding (8 cores): 2 m-shards x 4 k-shards. Each core handles
M_C=4096 rows of x and K_C=1024 output features.

Host prep (layout only): x is pre-blocked per m-shard into
[MT, 128(n%128), NCH*128(m)] fp32 so the device needs no transpose;
W_q is sent as bf16 (values 0..15 are exact); scales/zeros/mu2 folded
into per-(k,group) affine coefficients s' = scales*mu2, b' = -zeros*s'.

Per-core dataflow:
  Phase 1 (once): DMA W bf16 tiles; dequant on DVE with fused
    tensor_scalar (W*s' + b'); PE-transpose 128x128 chunks into the
    resident W.T [n%128, n//128, k] bf16 operand, folding mu1[n] in
    during the PSUM evict (alternating DVE/ACT to spread load).
  Phase 2 (streamed): SWDGE cast-DMA blocked x tiles fp32->bf16 on the
    Pool queue (prefetches during phase 1), then 32 accumulating bf16
    matmuls per (m-tile, k-tile) PSUM bank; bias added on PSUM evict.
"""

import sys

if "/opt/trn_rl_repo" not in sys.path:
    sys.path.insert(0, "/opt/trn_rl_repo")

import numpy as np
from ml_dtypes import bfloat16

import concourse.bass as bass  # noqa: F401
import concourse.mybir as mybir
import concourse.tile as tile
from concourse import bacc
from concourse.bass_utils import run_bass_kernel_spmd
from concourse.masks import make_identity

K, N = 4096, 4096
GROUP = 128
NG = N // GROUP  # 32 groups along N (group == 128-chunk)
M = 8192  # B*S
B, S = 4, 2048
M_SH, K_SH = 2, 4  # core grid: 2 m-shards x 4 k-shards
MC = M // M_SH  # 4096 rows per core
KC = K // K_SH  # 1024 output features per core
NCH = N // 128  # 32 contraction chunks
MT = MC // 128  # 32 m-tiles
KT = KC // 128  # 8 k-row-tiles of W
KTILE = 512  # matmul free dim (one PSUM bank)
NKT = KC // KTILE  # 2

_CACHE: dict = {}


def build_nc(
    repeat: int = 1,
    debug: bool = False,
    probe: str = "full",
    pso_bufs: int = 4,
    ilv: bool = True,
):
    """probe: 'full' | 'mm_only' (fixed x tile in repeat body) |
    'xprep_only' (no matmuls in repeat body).
    ilv: interleave the two kt2 PSUM groups g-major so consecutive
    matmuls share the same stationary operand (halves LDWEIGHTS)."""
    dt = mybir.dt
    nc = bacc.Bacc("TRN2", target_bir_lowering=False, debug=debug)

    # x blocked on host: [MT, 128 (n%128), NCH*128 (g-major, m-minor)]
    x_d = nc.dram_tensor("x", [MT, 128, N], dt.float32, kind="ExternalInput")
    wq_d = nc.dram_tensor("wq", [KC, N], dt.bfloat16, kind="ExternalInput")
    seff_d = nc.dram_tensor("seff", [KC, NG], dt.float32, kind="ExternalInput")
    beff_d = nc.dram_tensor("beff", [KC, NG], dt.float32, kind="ExternalInput")
    mu1_d = nc.dram_tensor("mu1t", [128, NG], dt.float32, kind="ExternalInput")
    bias_d = nc.dram_tensor("biasb", [128, KC], dt.float32, kind="ExternalInput")
    out_d = nc.dram_tensor("out", [MC, KC], dt.float32, kind="ExternalOutput")

    with tile.TileContext(nc) as tc:
        with (
            tc.tile_pool(name="const", bufs=1) as cpool,
            tc.tile_pool(name="xt", bufs=3) as xt_pool,
            tc.tile_pool(
                name="pso", bufs=(2 if ilv else pso_bufs), space="PSUM"
            ) as pso_pool,
            tc.tile_pool(name="osb", bufs=4) as osb_pool,
            tc.tile_pool(name="wq_pool", bufs=2) as wq_pool,
            tc.tile_pool(name="wstage", bufs=8) as ws_pool,
            tc.tile_pool(
                name="psw",
                bufs=(4 if ilv else 8 - pso_bufs),
                space="PSUM",
            ) as psw_pool,
        ):
            # seff/beff gate the first dequant: issue them FIRST on the SP
            # queue (ahead of the big W DMAs); mu1/bias (needed later) go on
            # the ACT HWDGE queue
            ident = cpool.tile([128, 128], dt.bfloat16)
            make_identity(nc, ident)
            seff_sb = cpool.tile([128, KT, NG], dt.float32)
            nc.sync.dma_start(
                out=seff_sb, in_=seff_d.rearrange("(t p) g -> p t g", p=128)
            )
            beff_sb = cpool.tile([128, KT, NG], dt.float32)
            nc.sync.dma_start(
                out=beff_sb, in_=beff_d.rearrange("(t p) g -> p t g", p=128)
            )
            mu1_sb = cpool.tile([128, NG], dt.float32)
            nc.scalar.dma_start(out=mu1_sb, in_=mu1_d[:, :])
            bias_sb = cpool.tile([128, NKT, KTILE], dt.float32)
            nc.scalar.dma_start(out=bias_sb, in_=bias_d[:, :])

            # Resident transposed weight operand, split per kt2 half:
            # wt[h][n % 128, n // 128, k_local] covers k = h*512 .. h*512+512
            wt_sb = [
                cpool.tile([128, NCH, KTILE], dt.bfloat16, name=f"wt_{h}")
                for h in range(NKT)
            ]

            # ---------------- Phase 1: dequant + transpose W ----------------
            for half in range(NKT):
                wq_tiles = []
                for i in range(4):
                    kt = half * 4 + i
                    wq_t = wq_pool.tile([128, N], dt.bfloat16, name=f"wq_{i}")
                    nc.sync.dma_start(
                        out=wq_t, in_=wq_d[kt * 128 : (kt + 1) * 128, :]
                    )
                    wq_tiles.append((kt, wq_t))
                for g2 in range(NG // 2):  # process groups in pairs
                    ps = psw_pool.tile([128, 2, KTILE], dt.bfloat16, name="psw")
                    for j in range(2):
                        g = g2 * 2 + j
                        stage = ws_pool.tile([128, 4, 128], dt.bfloat16, name="wstg")
                        for i, (kt, wq_t) in enumerate(wq_tiles):
                            # (Q * s') + b'  with s' = scales*mu2, b' = -z*s*mu2
                            # split across DVE (3/4) and Pool (1/4)
                            eng = nc.gpsimd if i == 3 else nc.vector
                            eng.tensor_scalar(
                                out=stage[:, i, :],
                                in0=wq_t[:, g * 128 : (g + 1) * 128],
                                scalar1=seff_sb[:, kt, g : g + 1],
                                scalar2=beff_sb[:, kt, g : g + 1],
                                op0=mybir.AluOpType.mult,
                                op1=mybir.AluOpType.add,
                            )
                        for i in range(4):
                            nc.tensor.transpose(
                                ps[:, j, i * 128 : (i + 1) * 128], stage[:, i, :], ident
                            )
                    # evict pair with mu1[n] fold (per-partition scalar);
                    # mu1 for both groups of the pair lives at mu1_sb[:, 2*g2:2*g2+2]
                    # but tensor ops need one scalar -> evict per group, split
                    # ACT (3/4) / DVE (1/4)
                    for j in range(2):
                        g = g2 * 2 + j
                        if g % 4 == 0:
                            nc.vector.tensor_scalar_mul(
                                out=wt_sb[half][:, g, :],
                                in0=ps[:, j, :],
                                scalar1=mu1_sb[:, g : g + 1],
                            )
                        else:
                            nc.scalar.activation(
                                out=wt_sb[half][:, g, :],
                                in_=ps[:, j, :],
                                func=mybir.ActivationFunctionType.Copy,
                                scale=mu1_sb[:, g : g + 1],
                            )

            # ---------------- Phase 2: stream x, matmul ----------------
            def x_load(mt, tag=""):
                # SWDGE cast-DMA: blocked fp32 DRAM -> bf16 SBUF, already
                # in [n%128, g, m] layout (host pre-blocked)
                xt_t = xt_pool.tile([128, NCH, 128], dt.bfloat16, name="xt" + tag)
                nc.gpsimd.dma_start(out=xt_t, in_=x_d[mt])
                return xt_t

            xt_fixed = x_load(0, tag="fix") if probe == "mm_only" else None
            for _rep in range(repeat):
                for mt in range(MT):
                    xt_t = xt_fixed if probe == "mm_only" else x_load(mt)
                    if probe == "xprep_only":
                        continue

                    def evict(pso, kt2, mt=mt):
                        osb = osb_pool.tile([128, KTILE], dt.float32, name="osb")
                        nc.vector.tensor_add(
                            out=osb,
                            in0=pso,
                            in1=bias_sb[:, kt2, :],
                        )
                        nc.sync.dma_start(
                            out=out_d[
                                mt * 128 : (mt + 1) * 128,
                                kt2 * KTILE : (kt2 + 1) * KTILE,
                            ],
                            in_=osb,
                        )

                    if ilv:
                        # one 2-bank PSUM tile; consecutive matmuls share the
                        # same stationary lhsT across the two kt2 banks
                        pso2 = pso_pool.tile([128, NKT, KTILE], dt.float32, name="pso2")
                        for g in range(NCH):
                            for kt2 in range(NKT):
                                nc.tensor.matmul(
                                    pso2[:, kt2, :],
                                    lhsT=xt_t[:, g, :],
                                    rhs=wt_sb[kt2][:, g, :],
                                    start=(g == 0),
                                    stop=(g == NCH - 1),
                                    skip_group_check=True,
                                )
                        osb = osb_pool.tile([128, NKT, KTILE], dt.float32, name="osb2")
                        nc.vector.tensor_add(out=osb, in0=pso2, in1=bias_sb)
                        nc.sync.dma_start(
                            out=out_d[mt * 128 : (mt + 1) * 128, :], in_=osb
                        )
                    else:
                        for kt2 in range(NKT):
                            pso = pso_pool.tile([128, KTILE], dt.float32, name="pso")
                            for g in range(NCH):
                                nc.tensor.matmul(
                                    pso,
                                    lhsT=xt_t[:, g, :],
                                    rhs=wt_sb[kt2][:, g, :],
                                    start=(g == 0),
                                    stop=(g == NCH - 1),
                                )
                            evict(pso, kt2)
    nc.compile()
    return nc


def make_in_maps(x, W_q, scales, zeros, mu1, mu2, bias):
    x2 = np.asarray(x, dtype=np.float32).reshape(M, N)
    W_q = np.asarray(W_q, dtype=np.int32)
    scales = np.asarray(scales, dtype=np.float32).reshape(K, NG)
    zeros = np.asarray(zeros, dtype=np.float32).reshape(K, NG)
    mu1 = np.asarray(mu1, dtype=np.float32)
    mu2 = np.asarray(mu2, dtype=np.float32)
    bias = np.asarray(bias, dtype=np.float32)

    s_eff = scales * mu2[:, None]  # [K, NG]
    b_eff = -(zeros * s_eff)  # [K, NG]
    mu1_t = np.ascontiguousarray(mu1.reshape(NG, 128).T)  # [128, NG]
    wq_bf = W_q.astype(bfloat16)  # values 0..15, exact in bf16

    # blocked x per m-shard: [MT, 128(n%128), NCH, 128(m)] -> [MT, 128, N]
    x_blk = []
    for mi in range(M_SH):
        xs = x2[mi * MC : (mi + 1) * MC]  # [MC, N]
        xb = xs.reshape(MT, 128, NCH, 128)  # [mt, m_l, g, p]
        xb = np.ascontiguousarray(xb.transpose(0, 3, 2, 1))  # [mt, p, g, m_l]
        x_blk.append(xb.reshape(MT, 128, N))

    in_maps = []
    for c in range(8):
        mi, ki = c // K_SH, c % K_SH
        in_maps.append(
            {
                "x": x_blk[mi],
                "wq": np.ascontiguousarray(wq_bf[ki * KC : (ki + 1) * KC]),
                "seff": np.ascontiguousarray(s_eff[ki * KC : (ki + 1) * KC]),
                "beff": np.ascontiguousarray(b_eff[ki * KC : (ki + 1) * KC]),
                "mu1t": mu1_t,
                "biasb": np.ascontiguousarray(
                    np.broadcast_to(bias[ki * KC : (ki + 1) * KC], (128, KC))
                ),
            }
        )
    return in_maps


def assemble(results):
    out = np.empty((M, K), np.float32)
    for c in range(8):
        mi, ki = c // K_SH, c % K_SH
        out[mi * MC : (mi + 1) * MC, ki * KC : (ki + 1) * KC] = results[c]["out"]
    return out.reshape(B, S, K)


def kernel(x, W_q, scales, zeros, mu1, mu2, bias):
    in_maps = make_in_maps(x, W_q, scales, zeros, mu1, mu2, bias)
    nc = _CACHE.get("nc")
    if nc is None:
        nc = build_nc()
        _CACHE["nc"] = nc
    res = run_bass_kernel_spmd(nc, in_maps, core_ids=list(range(8)))
    return assemble(res.results)
